# revision 6
# baseline (speedup 1.0000x reference)
"""Trainium2 Bass kernel for nn_BiLinearInteractionLayer.

Math: x:(B=4096, F=32, D=64) f32, W:(P=496, D=64, D=64) f32 (torch Linear
layout: out_e = sum_d in_d * W[e, d]).  For each pair p=(i,j), i<j:
    out[b, p, e] = (sum_d x[b,i,d] * W[p,e,d]) * x[b,j,e]

Strategy (data-parallel over batch, 8 cores x 512 rows):

The harness gate is rel_err < 2e-2 (normalized by the global max), so the
whole pipeline runs in fp16: single-pass fp16 matmuls (fp32 PSUM
accumulate), fp16 elementwise multiply, fp16 output stores that the host
widens back to f32 (measured ~8e-4 rel err).  Versus the fp32-exact
baseline this halves PE matmul passes, removes the hi/lo split entirely,
and halves HBM store traffic (the dominant cost: output is 65MB/core in
f32, 32.5MB in fp16).

All matmuls are k=128 even though the contraction is only 64 deep: the
stationary operand's rows 64-127 are zeros (memset once) and the moving
weight operand's rows 64-127 are an SBUF->SBUF copy of rows 0-63 (finite,
so 0*x can't produce NaN; copied on-chip so HBM traffic doesn't grow).
k=64 matmuls under-report to the HAM activity monitor and the PE then
never un-throttles from 1.2 GHz; with k=128 the PE reaches 2.4 GHz
(measured on the fp32 baseline: PE active 300us -> 177us).

Host preformatting: x ships twice in fp16 - natively (BL, F*D) for the
elementwise right-field operand, and pre-transposed (64, bt*F*128) for
the matmul stationary operand (loaded once into a persistent [128,16K]
tile) - plus W transposed to wt[d, p*64+e] in fp16.  No on-chip
transposes: the PE runs nothing but the pair matmuls.

Per 128-row batch tile, per left field i: the npair (<= 31) pair matmuls
go in bank-aligned chunks of <= 8 pairs into ONE 4-bank PSUM tile
(npair*64 f32 <= 7936B), then a SINGLE per-field evict / elementwise
multiply amortizes fixed per-instruction costs (~260ns/instr on ACT).
The combine is routed per field across three paths, balanced at build
time with trace-measured per-element rates:
  D: DVE tensor_mul direct from PSUM (1x: fp32 operand)
  A: ACT evicts PSUM->SBUF fp16, DVE tensor_mul all-fp16/SBUF (2x_1p)
  P: ACT evicts, GPSIMD does the multiply (GPSIMD has no PSUM port)
Outputs accumulate in per-group (4 left fields) fp16 tiles and store once
per group: 8 stores/bt with 4-15KB contiguous runs per partition.

DMA queue split: loads issue on the Activation HWDGE ring, stores on the
Sync ring, weight-duplication copies on GPSIMD SWDGE.  Sharing one ring
made batch-tile N's store semaphore wait block batch-tile N+1's loads
(in-order queue) and broke the inter-tile pipeline.

HBM traffic/core: 32.5MB out + 4MB wt + 4MB x (2 layouts) = 40.5MB.
"""
import numpy as np

import concourse.bacc as bacc
import concourse.tile as tile
import concourse.mybir as mybir
from concourse.bass_utils import run_bass_kernel_spmd

B = 4096
F = 32
D = 64
P = F * (F - 1) // 2  # 496
N_CORES = 8
BL = B // N_CORES     # 512 rows per core
BT = 128              # batch tile (SBUF partitions)
NBT = BL // BT        # 4 batch tiles per core
CHUNK = 8             # pairs per matmul chunk (8*64 = 512 = one PSUM bank)
TGROUP = 4            # left fields per output-store group
NLEFT = F - 1         # left fields 0..30

f32 = mybir.dt.float32
f16 = mybir.dt.float16

_nc_cache = None


def _off(i):
    """Pair index of the first pair with left field i."""
    return 31 * i - i * (i - 1) // 2


def _chunks(npair):
    # bank-aligned: chunk c starts at pair offset 8*c so every chunk's
    # f32 PSUM region stays inside one 2KB bank
    return [(c0, min(CHUNK, npair - c0)) for c0 in range(0, npair, CHUNK)]


_GROUPS = [(g0, min(TGROUP, NLEFT - g0)) for g0 in range(0, NLEFT, TGROUP)]

# trace-measured per-instruction engine costs: ns/elem (per lane), fixed ns
_ACT_RATE, _ACT_FIX = 0.833, 263.0
_DVE1_RATE, _DVE1_FIX = 1.042, 200.0   # tensor_tensor with PSUM f32 operand
_DVE2_RATE, _DVE2_FIX = 0.521, 175.0   # tensor_tensor all-SBUF fp16 (2x_1p)
_POOL_RATE, _POOL_FIX = 1.80, 250.0    # gpsimd tensor_tensor
_DMA_ISSUE_ACT = 667.0                 # HWDGE issue cost on the ACT queue


class _Balancer:
    """Greedy per-field route chooser minimizing the max engine load."""

    def __init__(self):
        self.act = 0.0
        self.dve = 0.0
        self.pool = 0.0

    def pick(self, e):
        cand = {
            "D": (0.0, _DVE1_RATE * e + _DVE1_FIX, 0.0),
            "A": (_ACT_RATE * e + _ACT_FIX, _DVE2_RATE * e + _DVE2_FIX, 0.0),
            "P": (_ACT_RATE * e + _ACT_FIX, 0.0, _POOL_RATE * e + _POOL_FIX),
        }
        best, best_load = None, None
        for r, (a, d, p) in cand.items():
            load = max(self.act + a, self.dve + d, self.pool + p)
            if best_load is None or load < best_load:
                best, best_load = r, load
        a, d, p = cand[best]
        self.act += a
        self.dve += d
        self.pool += p
        return best


def _build():
    nc = bacc.Bacc("TRN2", target_bir_lowering=False, debug=False,
                   num_devices=N_CORES)
    x_in = nc.dram_tensor("x", [BL, F * D], f16, kind="ExternalInput").ap()
    # xt[d, ((bt*F + f)*BT + r)] = x[bt*BT + r, f, d]
    xt_in = nc.dram_tensor("xt", [D, NBT * F * BT], f16,
                           kind="ExternalInput").ap()
    # wt[d, p*D + e] = W[p, e, d]
    wt_in = nc.dram_tensor("wt", [D, P * D], f16, kind="ExternalInput").ap()
    out = nc.dram_tensor("out", [BL, P * D], f16, kind="ExternalOutput").ap()

    bal = _Balancer()

    with tile.TileContext(nc) as tc:
        with (
            tc.tile_pool(name="consts", bufs=1) as consts,
            tc.tile_pool(name="xp", bufs=2) as xp,
            tc.tile_pool(name="otp", bufs=3) as otp,
            tc.tile_pool(name="pm16p", bufs=4) as pm16p,
            tc.tile_pool(name="psm", bufs=2, space="PSUM") as psm,
        ):
            # persistent transposed-x: rows 0-63 real data (one 2MB load),
            # rows 64-127 zeros so k=128 matmuls are exact
            xt_all = consts.tile([2 * D, NBT * F * BT], f16, tag="xta")
            nc.vector.memset(xt_all[D:2 * D, :NBT * F * BT // 2], 0.0)
            nc.gpsimd.memset(xt_all[D:2 * D, NBT * F * BT // 2:], 0.0)
            bal.dve += NBT * F * BT // 2 * _DVE1_RATE
            bal.pool += NBT * F * BT // 2 * 0.833
            nc.scalar.dma_start(out=xt_all[0:D, :], in_=xt_in)
            bal.act += _DMA_ISSUE_ACT

            # per-group weight tiles: rows 0-63 from HBM, rows 64-127 an
            # on-chip copy (multiplied by the xt zeros; must be finite)
            wt_g = []
            for gi, (g0, gn) in enumerate(_GROUPS):
                c0 = _off(g0) * D
                c1 = _off(g0 + gn) * D
                t = consts.tile([2 * D, c1 - c0], f16, tag=f"wt{gi}")
                wt_g.append(t)
            for gi, (g0, gn) in enumerate(_GROUPS):
                c0 = _off(g0) * D
                c1 = _off(g0 + gn) * D
                nc.scalar.dma_start(out=wt_g[gi][0:D, :], in_=wt_in[:, c0:c1])
                bal.act += _DMA_ISSUE_ACT
                nc.gpsimd.dma_start(out=wt_g[gi][D:2 * D, :],
                                    in_=wt_g[gi][0:D, :])
                bal.pool += 1000.0

            for bt in range(NBT):
                rows = slice(bt * BT, (bt + 1) * BT)
                x_tile = xp.tile([BT, F * D], f16, tag="x")
                nc.scalar.dma_start(out=x_tile, in_=x_in[rows, :])
                bal.act += _DMA_ISSUE_ACT

                for gi, (g0, gn) in enumerate(_GROUPS):
                    gbase = _off(g0) * D
                    gsz = (_off(g0 + gn) - _off(g0)) * D
                    ot = otp.tile([BT, gsz], f16, tag="ot")
                    for i in range(g0, g0 + gn):
                        npair = F - 1 - i  # pairs (i, i+1..31), consecutive
                        p0 = _off(i)
                        n_i = npair * D
                        lhsT = xt_all[:, (bt * F + i) * BT:
                                      (bt * F + i + 1) * BT]  # [128, 128]
                        # one 4-bank PSUM tile holds the whole field
                        pm = psm.tile([BT, 4 * CHUNK * D], f32, tag="mm")
                        for c0, cn in _chunks(npair):
                            n = cn * D
                            ws = (p0 + c0) * D - gbase
                            nc.tensor.matmul(
                                pm[:, c0 * D:c0 * D + n], lhsT,
                                wt_g[gi][:, ws:ws + n],
                                start=True, stop=True)
                        xj = x_tile[:, (i + 1) * D:(i + 1) * D + n_i]
                        ot_sl = ot[:, (p0 * D - gbase):(p0 * D - gbase) + n_i]
                        route = bal.pick(n_i)
                        if route == "D":
                            nc.vector.tensor_mul(ot_sl, pm[:, :n_i], xj)
                        else:
                            pm16 = pm16p.tile([BT, n_i], f16, tag="pm16")
                            nc.scalar.copy(pm16, pm[:, :n_i])
                            if route == "A":
                                nc.vector.tensor_mul(ot_sl, pm16, xj)
                            else:
                                nc.gpsimd.tensor_mul(ot_sl, pm16, xj)
                    nc.sync.dma_start(out=out[rows, gbase:gbase + gsz],
                                      in_=ot)
    nc.compile()
    return nc


def _get_nc():
    global _nc_cache
    if _nc_cache is None:
        _nc_cache = _build()
    return _nc_cache


def _prep_inputs(x, W):
    x16 = np.asarray(x, dtype=np.float16).reshape(N_CORES, BL, F * D)
    # per-core pre-transposed layout: [D, NBT, F, BT] flattened
    xt = np.ascontiguousarray(
        x16.reshape(N_CORES, NBT, BT, F, D).transpose(0, 4, 1, 3, 2)
    ).reshape(N_CORES, D, NBT * F * BT)
    wt = np.ascontiguousarray(
        np.asarray(W, dtype=np.float32).transpose(2, 0, 1)
    ).reshape(D, P * D).astype(np.float16)
    x16 = np.ascontiguousarray(x16)
    return x16, xt, wt


def _run(x, W, trace=False, trace_kwargs=None):
    x16, xt, wt = _prep_inputs(x, W)
    in_maps = [{"x": x16[c], "xt": xt[c], "wt": wt}
               for c in range(N_CORES)]
    res = run_bass_kernel_spmd(_get_nc(), in_maps, list(range(N_CORES)),
                               trace=trace, **(trace_kwargs or {}))
    outs = [res.results[c]["out"].reshape(BL, P, D) for c in range(N_CORES)]
    return np.concatenate(outs, axis=0).astype(np.float32), res


def kernel(x, W):
    out, _ = _run(x, W)
    return out


# revision 8
# speedup vs baseline: 1.2602x; 1.2602x over previous
"""Trainium2 Bass kernel for nn_BiLinearInteractionLayer.

Math: x:(B=4096, F=32, D=64) f32, W:(P=496, D=64, D=64) f32 (torch Linear
layout: out_e = sum_d in_d * W[e, d]).  For each pair p=(i,j), i<j:
    out[b, p, e] = (sum_d x[b,i,d] * W[p,e,d]) * x[b,j,e]

Strategy (data-parallel over batch, 8 cores x 512 rows):

The harness gate is rel_err < 2e-2 (normalized by the global max), so the
whole pipeline runs in fp16: single-pass fp16 matmuls (fp32 PSUM
accumulate), fp16 elementwise multiply, fp16 output stores that the host
widens back to f32 (measured ~8e-4 rel err).  Versus the fp32-exact
baseline this halves PE matmul passes, removes the hi/lo split entirely,
and halves HBM store traffic (the dominant cost: output is 65MB/core in
f32, 32.5MB in fp16).

All matmuls are k=128 even though the contraction is only 64 deep: the
host ships the transposed x duplicated across both partition halves and
the weights HALVED and duplicated (x^T*(W/2) + x^T*(W/2) = x^T*W, exact
since /2 is a power of two).  k=64 matmuls under-report to the HAM
activity monitor and the PE never un-throttles from 1.2 GHz; with real
(not zero-padded: tried, boost was intermittent) k=128 data the PE holds
2.4 GHz.  Duplication costs +6MB/core of load traffic but removes the
on-chip zero-fill/copy serialization that stalled the first ~40us.

Host preformatting: x ships natively in fp16 (BL, F*D) for the
elementwise right-field operand; x^T duplicated (128, bt*F*128) for the
matmul stationary operand (persistent SBUF tile, loaded per-bt-slice so
the first matmul only waits on 1MB); W as wt[d, p*64+e]/2 duplicated
(128, P*64).  No on-chip transposes: the PE runs nothing but matmuls.

Per 128-row batch tile, per left field i: pair matmuls go in bank-
aligned chunks of <= 8 pairs into 2-bank PSUM tiles (<= 16 pairs each,
bufs=4 for PE run-ahead), then per-subfield evict / elementwise multiply
(amortizes the ~300ns fixed cost per instruction vs chunk granularity).
The combine is routed per subfield across three paths, balanced at build
time with trace-measured per-element rates:
  D: DVE tensor_mul direct from PSUM (1x mode: fp32 operand)
  A: ACT evicts PSUM->SBUF fp16, DVE tensor_mul all-fp16/SBUF
  P: ACT evicts, GPSIMD does the multiply (GPSIMD has no PSUM port)
Outputs accumulate in per-group (4 left fields) fp16 tiles and store once
per group: 8 stores/bt with 4-15KB contiguous runs per partition.

DMA queue split: loads issue on the Activation HWDGE ring in
first-needed order, stores on the Sync ring.  Sharing one ring made
batch-tile N's store semaphore wait block batch-tile N+1's loads
(in-order queue) and broke the inter-tile pipeline.

HBM traffic/core: 32.5MB out + 8MB wt + 4MB xt + 2MB x = 46.5MB.
"""
import numpy as np

import concourse.bacc as bacc
import concourse.tile as tile
import concourse.mybir as mybir
from concourse.bass_utils import run_bass_kernel_spmd

B = 4096
F = 32
D = 64
P = F * (F - 1) // 2  # 496
N_CORES = 8
BL = B // N_CORES     # 512 rows per core
BT = 128              # batch tile (SBUF partitions)
NBT = BL // BT        # 4 batch tiles per core
CHUNK = 8             # pairs per matmul chunk (8*64 = 512 = one PSUM bank)
SUBF = 16             # pairs per PSUM tile / combine instruction (2 banks)
TGROUP = 4            # left fields per output-store group
NLEFT = F - 1         # left fields 0..30

f32 = mybir.dt.float32
f16 = mybir.dt.float16

_nc_cache = None


def _off(i):
    """Pair index of the first pair with left field i."""
    return 31 * i - i * (i - 1) // 2


_GROUPS = [(g0, min(TGROUP, NLEFT - g0)) for g0 in range(0, NLEFT, TGROUP)]

# trace-measured per-instruction engine costs: ns/elem (per lane), fixed ns
_ACT_RATE, _ACT_FIX = 0.87, 300.0
_DVE1_RATE, _DVE1_FIX = 1.042, 200.0   # tensor_tensor with PSUM f32 operand
_DVE2_RATE, _DVE2_FIX = 0.75, 200.0    # tensor_tensor all-SBUF fp16
_POOL_RATE, _POOL_FIX = 1.80, 330.0    # gpsimd tensor_tensor
_DMA_ISSUE_ACT = 620.0                 # HWDGE issue cost on the ACT queue


class _Balancer:
    """Greedy per-subfield route chooser minimizing the max engine load."""

    def __init__(self):
        self.act = 0.0
        self.dve = 0.0
        self.pool = 0.0

    def pick(self, e):
        cand = {
            "D": (0.0, _DVE1_RATE * e + _DVE1_FIX, 0.0),
            "A": (_ACT_RATE * e + _ACT_FIX, _DVE2_RATE * e + _DVE2_FIX, 0.0),
            "P": (_ACT_RATE * e + _ACT_FIX, 0.0, _POOL_RATE * e + _POOL_FIX),
        }
        best, best_load = None, None
        for r, (a, d, p) in cand.items():
            load = max(self.act + a, self.dve + d, self.pool + p)
            if best_load is None or load < best_load:
                best, best_load = r, load
        a, d, p = cand[best]
        self.act += a
        self.dve += d
        self.pool += p
        return best


def _build():
    nc = bacc.Bacc("TRN2", target_bir_lowering=False, debug=False,
                   num_devices=N_CORES)
    x_in = nc.dram_tensor("x", [BL, F * D], f16, kind="ExternalInput").ap()
    # xt[d, ((bt*F + f)*BT + r)] = x[bt*BT + r, f, d], rows 64-127 duplicate
    xt_in = nc.dram_tensor("xt", [2 * D, NBT * F * BT], f16,
                           kind="ExternalInput").ap()
    # wt[d, p*D + e] = W[p, e, d] / 2, rows 64-127 duplicate rows 0-63
    wt_in = nc.dram_tensor("wt", [2 * D, P * D], f16,
                           kind="ExternalInput").ap()
    out = nc.dram_tensor("out", [BL, P * D], f16, kind="ExternalOutput").ap()

    bal = _Balancer()

    with tile.TileContext(nc) as tc:
        with (
            tc.tile_pool(name="consts", bufs=1) as consts,
            tc.tile_pool(name="xp", bufs=2) as xp,
            tc.tile_pool(name="otp", bufs=3) as otp,
            tc.tile_pool(name="pm16p", bufs=6) as pm16p,
            tc.tile_pool(name="psm", bufs=4, space="PSUM") as psm,
        ):
            # persistent transposed-x (both halves real, duplicated)
            xt_all = consts.tile([2 * D, NBT * F * BT], f16, tag="xta")
            wt_g = []
            for gi, (g0, gn) in enumerate(_GROUPS):
                c0 = _off(g0) * D
                c1 = _off(g0 + gn) * D
                t = consts.tile([2 * D, c1 - c0], f16, tag=f"wt{gi}")
                wt_g.append(t)

            def load_xt(bt):
                sl = slice(bt * F * BT, (bt + 1) * F * BT)
                nc.scalar.dma_start(out=xt_all[:, sl], in_=xt_in[:, sl])
                bal.act += _DMA_ISSUE_ACT

            def load_wt(gi):
                c0 = _off(_GROUPS[gi][0]) * D
                c1 = _off(_GROUPS[gi][0] + _GROUPS[gi][1]) * D
                nc.scalar.dma_start(out=wt_g[gi], in_=wt_in[:, c0:c1])
                bal.act += _DMA_ISSUE_ACT

            x_tiles = {}

            def load_x(bt):
                xt_ = xp.tile([BT, F * D], f16, tag="x")
                nc.scalar.dma_start(
                    out=xt_, in_=x_in[bt * BT:(bt + 1) * BT, :])
                bal.act += _DMA_ISSUE_ACT
                x_tiles[bt] = xt_

            # first-needed order: bt0's transposed slice and first weight
            # groups lead; later bt slices interleave behind
            load_xt(0)
            load_wt(0)
            load_wt(1)
            load_x(0)
            for gi in range(2, len(_GROUPS)):
                load_wt(gi)
            load_xt(1)

            for bt in range(NBT):
                rows = slice(bt * BT, (bt + 1) * BT)
                if bt + 2 < NBT:
                    load_xt(bt + 2)
                if bt + 1 < NBT:
                    load_x(bt + 1)
                x_tile = x_tiles.pop(bt)

                for gi, (g0, gn) in enumerate(_GROUPS):
                    gbase = _off(g0) * D
                    gsz = (_off(g0 + gn) - _off(g0)) * D
                    ot = otp.tile([BT, gsz], f16, tag="ot")
                    for i in range(g0, g0 + gn):
                        npair = F - 1 - i  # pairs (i, i+1..31), consecutive
                        p0 = _off(i)
                        lhsT = xt_all[:, (bt * F + i) * BT:
                                      (bt * F + i + 1) * BT]  # [128, 128]
                        for s0 in range(0, npair, SUBF):
                            sn = min(SUBF, npair - s0)
                            n_s = sn * D
                            # one 2-bank PSUM tile per <=16-pair subfield
                            pm = psm.tile([BT, 2 * CHUNK * D], f32, tag="mm")
                            for c0 in range(0, sn, CHUNK):
                                cn = min(CHUNK, sn - c0)
                                n = cn * D
                                ws = (p0 + s0 + c0) * D - gbase
                                nc.tensor.matmul(
                                    pm[:, c0 * D:c0 * D + n], lhsT,
                                    wt_g[gi][:, ws:ws + n],
                                    start=True, stop=True)
                            j0 = (i + 1 + s0) * D
                            xj = x_tile[:, j0:j0 + n_s]
                            osl = (p0 + s0) * D - gbase
                            ot_sl = ot[:, osl:osl + n_s]
                            route = bal.pick(n_s)
                            if route == "D":
                                nc.vector.tensor_mul(ot_sl, pm[:, :n_s], xj)
                            else:
                                pm16 = pm16p.tile([BT, n_s], f16, tag="pm16")
                                nc.scalar.copy(pm16, pm[:, :n_s])
                                if route == "A":
                                    nc.vector.tensor_mul(ot_sl, pm16, xj)
                                else:
                                    nc.gpsimd.tensor_mul(ot_sl, pm16, xj)
                    nc.sync.dma_start(out=out[rows, gbase:gbase + gsz],
                                      in_=ot)
    nc.compile()
    return nc


def _get_nc():
    global _nc_cache
    if _nc_cache is None:
        _nc_cache = _build()
    return _nc_cache


def _prep_inputs(x, W):
    x16 = np.asarray(x, dtype=np.float16).reshape(N_CORES, BL, F * D)
    # per-core pre-transposed layout: [D, NBT, F, BT] flattened, duplicated
    xt1 = np.ascontiguousarray(
        x16.reshape(N_CORES, NBT, BT, F, D).transpose(0, 4, 1, 3, 2)
    ).reshape(N_CORES, D, NBT * F * BT)
    xt = np.concatenate([xt1, xt1], axis=1)  # (N_CORES, 128, ...)
    wt1 = (np.ascontiguousarray(
        np.asarray(W, dtype=np.float32).transpose(2, 0, 1)
    ).reshape(D, P * D) * 0.5).astype(np.float16)
    wt = np.concatenate([wt1, wt1], axis=0)  # (128, P*D)
    x16 = np.ascontiguousarray(x16)
    xt = np.ascontiguousarray(xt)
    return x16, xt, wt


def _run(x, W, trace=False, trace_kwargs=None):
    x16, xt, wt = _prep_inputs(x, W)
    in_maps = [{"x": x16[c], "xt": xt[c], "wt": wt}
               for c in range(N_CORES)]
    res = run_bass_kernel_spmd(_get_nc(), in_maps, list(range(N_CORES)),
                               trace=trace, **(trace_kwargs or {}))
    outs = [res.results[c]["out"].reshape(BL, P, D) for c in range(N_CORES)]
    return np.concatenate(outs, axis=0).astype(np.float32), res


def kernel(x, W):
    out, _ = _run(x, W)
    return out
